# revision 2
# baseline (speedup 1.0000x reference)
# Multi-head causal self-attention (B=4, T=2048, C=1024, H=16) on 8 NeuronCores.
#
# Sharding: core c handles batch b = c//2 and head-group g = c%2 (8 heads each).
# Each core computes q/k/v projections for its 8 heads, causal flash-style
# attention, and a partial out-projection (its 512 rows of w_out). The host
# sums the two partial outputs per batch (the 2-way tensor-parallel
# all-reduce) and concatenates batches.
#
# On-device layout choices (all matmuls run as fp32r = fp22-truncated fp32,
# full PE rate at moving dim >= 256):
#   - x is pre-transposed on host: xt [C, T] so the C-contraction of the
#     projections has C on SBUF partitions.
#   - q,k are produced feature-major (qT/kT [64*heads, T]) so scores can be
#     computed transposed: S^T [T_k, T_q] = kT.T @ qT, K=64, two heads packed
#     into the PE array via row tile_position (0/64).
#   - softmax runs without max-subtraction (scores are O(5), exp is safe in
#     fp32): P^T = exp(S^T/8) via ScalarE directly PSUM->SBUF.
#   - v is produced natural ([T, feats]) into v_pad with a ones-column per
#     128-key tile, so the PV matmul (V|1).T @ P^T accumulates both attn^T and
#     the softmax denominators in one pass: psum [65, T_q].
#   - causal masking: per diagonal 128-key tile only cols >= 128*d are
#     computed (restricted matmul output range); the remaining 128/256-wide
#     boundary block is masked by one DVE multiply with a host-built
#     triangular mask.
#   - normalization is fused into the PSUM->SBUF copy of attn^T (multiply by
#     the partition-broadcast reciprocal of the denominator row).
#   - out_proj: out[T,1024] partial = attnT.T @ w_out[512 rows], accumulated
#     over 4 feature tiles.

import numpy as np

_B, _T, _C = 4, 2048, 1024
_H, _DK = 16, 64
N_CORES = 8
_F = 512            # local features per core (8 heads x 64)
_NCT = _C // 128    # 8 C tiles
_NKT = _T // 128    # 16 key tiles
_NCH = _T // 512    # 4 query chunks of 512
_VPH = _NKT * 65    # 1040 v_pad cols per head


def _emit_body(nc, tc, mybir, xt, wqk, wv, wout, mask, out):
    """Emit one full forward pass. APs are the DRAM tensors."""
    F32R = mybir.dt.float32r
    F32 = mybir.dt.float32
    AF = mybir.ActivationFunctionType

    with tc.tile_pool(name="persist", bufs=1) as pp:
        mask_sb = pp.tile([128, 256], F32, name="mask_sb")
        nc.sync.dma_start(mask_sb[:], mask)

        qkT = [pp.tile([128, _T], F32R, name=f"qkT{fg}") for fg in range(8)]
        v_pad = pp.tile([128, 8 * _VPH], F32R, name="v_pad")
        v_pad_h = v_pad[:].rearrange("p (h c) -> p h c", h=8)

        # ones columns of v_pad (col 64 of each 65-wide key-tile slot)
        v_pad_slots = v_pad[:].rearrange("p (h t c) -> p h t c", h=8, t=_NKT)
        for h in range(8):
            nc.vector.memset(v_pad_slots[:, h, :, 64].bitcast(F32), 1.0)

        # ---------------- Phase A: projections (T halved to fit SBUF) ------
        with tc.tile_pool(name="wpool", bufs=1) as wp, \
             tc.tile_pool(name="xtpool", bufs=10) as xp, \
             tc.tile_pool(name="papsum", bufs=4, space="PSUM") as pap:
            wqk_sb = [wp.tile([128, 1024], F32R, name=f"wqk{kc}") for kc in range(_NCT)]
            wv_sb = [wp.tile([128, _F], F32R, name=f"wv{kc}") for kc in range(_NCT)]
            for kc in range(_NCT):
                nc.sync.dma_start(wqk_sb[kc][:], wqk[128 * kc:128 * (kc + 1), :])
                nc.sync.dma_start(wv_sb[kc][:], wv[128 * kc:128 * (kc + 1), :])

            for half in range(2):
                xt_sb = []
                for kc in range(_NCT):
                    t = xp.tile([128, 1024], F32R, tag="xt", bufs=10, name=f"xt{half}_{kc}")
                    nc.sync.dma_start(
                        t[:], xt[128 * kc:128 * (kc + 1), 1024 * half:1024 * (half + 1)])
                    xt_sb.append(t)

                # qT / kT, feature-major: psum [128 feats, 512 T]
                for fg in range(8):
                    for chl in range(2):
                        ch = 2 * half + chl
                        ps = pap.tile([128, 512], F32, tag="pa", bufs=4, name=f"psqk{fg}_{ch}")
                        for kc in range(_NCT):
                            nc.tensor.matmul(
                                ps[:],
                                wqk_sb[kc][:, 128 * fg:128 * (fg + 1)],
                                xt_sb[kc][:, 512 * chl:512 * (chl + 1)],
                                start=(kc == 0), stop=(kc == _NCT - 1))
                        nc.vector.tensor_copy(qkT[fg][:, 512 * ch:512 * (ch + 1)], ps[:])

                # v natural: psum [128 T, 512 feats] -> strided into v_pad
                for ttl in range(8):
                    tt = 8 * half + ttl
                    ps = pap.tile([128, 512], F32, tag="pa", bufs=4, name=f"psv{tt}")
                    for kc in range(_NCT):
                        nc.tensor.matmul(
                            ps[:],
                            xt_sb[kc][:, 128 * ttl:128 * (ttl + 1)],
                            wv_sb[kc][:],
                            start=(kc == 0), stop=(kc == _NCT - 1))
                    nc.vector.tensor_copy(
                        v_pad_h[:, :, 65 * tt:65 * tt + 64],
                        ps[:].rearrange("p (h c) -> p h c", h=8))

        # ---------------- Phases B+C: attention + out_proj -----------------
        with tc.tile_pool(name="atpool", bufs=1) as ap_, \
             tc.tile_pool(name="wopool", bufs=1) as wop, \
             tc.tile_pool(name="ptpool", bufs=2) as ptp, \
             tc.tile_pool(name="bcpool", bufs=2) as bcp, \
             tc.tile_pool(name="ostpool", bufs=3) as osp, \
             tc.tile_pool(name="scpsum", bufs=3, space="PSUM") as scp, \
             tc.tile_pool(name="pvpsum", bufs=2, space="PSUM") as pvp, \
             tc.tile_pool(name="popsum", bufs=1, space="PSUM") as pop:

            attnT = [ap_.tile([128, _T], F32R, name=f"attnT{p}") for p in range(4)]
            wout_sb = [wop.tile([128, 1024], F32R, name=f"wout{f}") for f in range(4)]
            for f in range(4):
                nc.sync.dma_start(wout_sb[f][:], wout[128 * f:128 * (f + 1), :])

            for j in range(_NCH):
                for p in range(4):
                    pvA = pvp.tile([65, 512], F32, tag="pvA", bufs=2, name=f"pvA{j}_{p}")
                    pvB = pvp.tile([65, 512], F32, tag="pvB", bufs=2, name=f"pvB{j}_{p}")
                    nkt = 4 * (j + 1)
                    for kt in range(nkt):
                        d = kt - 4 * j
                        if d < 0:
                            col_lo = 0
                        elif d < 3:
                            col_lo = 128 * d
                        else:
                            col_lo = 256
                        q_sl = slice(512 * j + col_lo, 512 * (j + 1))
                        c_sl = slice(col_lo, 512)
                        k_sl = slice(128 * kt, 128 * (kt + 1))

                        psA = scp.tile([128, 512], F32, tag="sc", bufs=3, name=f"psA{j}_{p}_{kt}")
                        psB = scp.tile([128, 512], F32, tag="sc", bufs=3, name=f"psB{j}_{p}_{kt}")
                        # S^T = kT.T @ qT, two heads row-packed (K=64)
                        nc.tensor.matmul(psA[:, c_sl], qkT[4 + p][0:64, k_sl],
                                         qkT[p][0:64, q_sl], start=True, stop=True)
                        nc.tensor.matmul(psB[:, c_sl], qkT[4 + p][64:128, k_sl],
                                         qkT[p][64:128, q_sl], start=True, stop=True)

                        pTA = ptp.tile([128, 512], F32R, tag="pTA", bufs=2, name=f"pTA{j}_{p}_{kt}")
                        pTB = ptp.tile([128, 512], F32R, tag="pTB", bufs=2, name=f"pTB{j}_{p}_{kt}")
                        nc.scalar.activation(pTA[:, c_sl], psA[:, c_sl], AF.Exp, scale=0.125)
                        nc.scalar.activation(pTB[:, c_sl], psB[:, c_sl], AF.Exp, scale=0.125)

                        if d >= 0:
                            if d < 3:
                                m_sl = slice(128 * d, 128 * (d + 1))
                                msk = mask_sb[:, 128:256]
                            else:
                                m_sl = slice(256, 512)
                                msk = mask_sb[:, 0:256]
                            nc.vector.tensor_mul(pTA[:, m_sl], pTA[:, m_sl], msk)
                            nc.vector.tensor_mul(pTB[:, m_sl], pTB[:, m_sl], msk)

                        # PV: (V|1).T @ P^T accumulated over key tiles
                        nc.tensor.matmul(pvA[0:65, c_sl],
                                         v_pad_h[:, 2 * p, 65 * kt:65 * kt + 65],
                                         pTA[:, c_sl],
                                         start=(kt == 0), stop=(kt == nkt - 1))
                        nc.tensor.matmul(pvB[0:65, c_sl],
                                         v_pad_h[:, 2 * p + 1, 65 * kt:65 * kt + 65],
                                         pTB[:, c_sl],
                                         start=(kt == 0), stop=(kt == nkt - 1))

                    # normalize: attnT rows = attn^T / denom
                    ch_sl = slice(512 * j, 512 * (j + 1))
                    recA = bcp.tile([1, 512], F32, tag="recA", bufs=2, name=f"recA{j}_{p}")
                    recB = bcp.tile([1, 512], F32, tag="recB", bufs=2, name=f"recB{j}_{p}")
                    nc.vector.reciprocal(recA[:], pvA[64:65, :])
                    nc.vector.reciprocal(recB[:], pvB[64:65, :])
                    bcA = bcp.tile([64, 512], F32, tag="bcA", bufs=2, name=f"bcA{j}_{p}")
                    bcB = bcp.tile([64, 512], F32, tag="bcB", bufs=2, name=f"bcB{j}_{p}")
                    nc.gpsimd.partition_broadcast(bcA[:], recA[:])
                    nc.gpsimd.partition_broadcast(bcB[:], recB[:])
                    nc.vector.tensor_mul(attnT[p][0:64, ch_sl], pvA[0:64, :], bcA[:])
                    nc.vector.tensor_mul(attnT[p][64:128, ch_sl], pvB[0:64, :], bcB[:])

                # out_proj for this chunk
                for mt in range(4):
                    t_sl = slice(512 * j + 128 * mt, 512 * j + 128 * (mt + 1))
                    ost = osp.tile([128, 1024], F32, tag="ost", bufs=3, name=f"ost{j}_{mt}")
                    for nh in range(2):
                        po = pop.tile([128, 512], F32, tag="po", bufs=1, name=f"po{j}_{mt}_{nh}")
                        for f in range(4):
                            nc.tensor.matmul(po[:],
                                             attnT[f][:, t_sl],
                                             wout_sb[f][:, 512 * nh:512 * (nh + 1)],
                                             start=(f == 0), stop=(f == 3))
                        nc.vector.tensor_copy(ost[:, 512 * nh:512 * (nh + 1)], po[:])
                    nc.sync.dma_start(out[t_sl, :], ost[:])


def build_program(repeat=1):
    import concourse.bass as bass  # noqa: F401
    import concourse.mybir as mybir
    import concourse.tile as tile
    from concourse import bacc

    F32R = mybir.dt.float32r
    F32 = mybir.dt.float32

    nc = bacc.Bacc("TRN2", target_bir_lowering=False, debug=False,
                   num_devices=N_CORES)
    xt = nc.dram_tensor("xt", [_C, _T], F32R, kind="ExternalInput").ap()
    wqk = nc.dram_tensor("wqk", [_C, 2 * _F], F32R, kind="ExternalInput").ap()
    wv = nc.dram_tensor("wv", [_C, _F], F32R, kind="ExternalInput").ap()
    wout = nc.dram_tensor("wout", [_F, _C], F32R, kind="ExternalInput").ap()
    mask = nc.dram_tensor("mask", [128, 256], F32, kind="ExternalInput").ap()
    out = nc.dram_tensor("out", [_T, _C], F32, kind="ExternalOutput").ap()

    with tile.TileContext(nc) as tc:
        if repeat == 1:
            _emit_body(nc, tc, mybir, xt, wqk, wv, wout, mask, out)
        else:
            with tc.For_i(0, repeat, 1):
                _emit_body(nc, tc, mybir, xt, wqk, wv, wout, mask, out)
    nc.compile()
    return nc


def make_in_maps(x, w_qkv, w_out):
    """Host-side sharding: per-core input dicts."""
    x = np.asarray(x, dtype=np.float32)
    w_qkv = np.asarray(w_qkv, dtype=np.float32)
    w_out = np.asarray(w_out, dtype=np.float32)

    mask_np = np.zeros((128, 256), np.float32)
    mask_np[:, 128:] = (np.arange(128)[:, None] <= np.arange(128)[None, :])

    xts = [np.ascontiguousarray(x[b].T) for b in range(_B)]
    in_maps = []
    for c in range(N_CORES):
        b, g = divmod(c, 2)
        qcols = w_qkv[:, 512 * g:512 * (g + 1)]
        kcols = w_qkv[:, _C + 512 * g:_C + 512 * (g + 1)]
        vcols = w_qkv[:, 2 * _C + 512 * g:2 * _C + 512 * (g + 1)]
        in_maps.append({
            "xt": xts[b],
            "wqk": np.ascontiguousarray(np.concatenate([qcols, kcols], axis=1)),
            "wv": np.ascontiguousarray(vcols),
            "wout": np.ascontiguousarray(w_out[512 * g:512 * (g + 1), :]),
            "mask": mask_np,
        })
    return in_maps


_PROGRAM_CACHE = {}


def _get_program(repeat=1):
    if repeat not in _PROGRAM_CACHE:
        _PROGRAM_CACHE[repeat] = build_program(repeat)
    return _PROGRAM_CACHE[repeat]


def kernel(x, w_qkv, w_out):
    from concourse.bass_utils import run_bass_kernel_spmd

    nc = _get_program(1)
    in_maps = make_in_maps(x, w_qkv, w_out)
    res = run_bass_kernel_spmd(nc, in_maps, core_ids=list(range(N_CORES)))
    out = np.empty((_B, _T, _C), np.float32)
    for b in range(_B):
        np.add(res.results[2 * b]["out"], res.results[2 * b + 1]["out"], out=out[b])
    return out


# revision 4
# speedup vs baseline: 1.1883x; 1.1883x over previous
# Multi-head causal self-attention (B=4, T=2048, C=1024, H=16) on 8 NeuronCores.
#
# Sharding: core c handles batch b = c//2 and head-group g = c%2 (8 heads each).
# Each core computes q/k/v projections for its 8 heads, causal flash-style
# attention, and a partial out-projection (its 512 rows of w_out). The host
# sums the two partial outputs per batch (the 2-way tensor-parallel
# all-reduce) and concatenates batches.
#
# On-device layout choices (all matmuls run as fp32r = fp22-truncated fp32,
# full PE rate at moving dim >= 256):
#   - x is pre-transposed on host: xt [C, T] so the C-contraction of the
#     projections has C on SBUF partitions.
#   - q,k are produced feature-major (qT/kT [64*heads, T]) so scores can be
#     computed transposed: S^T [T_k, T_q] = kT.T @ qT, K=64, two heads packed
#     into the PE array via row tile_position (0/64).
#   - softmax runs without max-subtraction (scores are O(5), exp is safe in
#     fp32): P^T = exp(S^T/8) via ScalarE directly PSUM->SBUF.
#   - v is produced natural ([T, feats]) into v_pad with a ones-column per
#     128-key tile, so the PV matmul (V|1).T @ P^T accumulates both attn^T and
#     the softmax denominators in one pass: psum [65, T_q].
#   - causal masking: per diagonal 128-key tile only cols >= 128*d are
#     computed (restricted matmul output range); the remaining 128/256-wide
#     boundary block is masked by one DVE multiply with a host-built
#     triangular mask.
#   - normalization is fused into the PSUM->SBUF copy of attn^T (multiply by
#     the partition-broadcast reciprocal of the denominator row).
#   - out_proj: out[T,1024] partial = attnT.T @ w_out[512 rows], accumulated
#     over 4 feature tiles.

import numpy as np

_B, _T, _C = 4, 2048, 1024
_H, _DK = 16, 64
N_CORES = 8
_F = 512            # local features per core (8 heads x 64)
_NCT = _C // 128    # 8 C tiles
_NKT = _T // 128    # 16 key tiles
_NCH = _T // 512    # 4 query chunks of 512
_VPH = _NKT * 65    # 1040 v_pad cols per head


def _emit_body(nc, tc, mybir, xt, wqk, wv, wout, mask, out):
    """Emit one full forward pass. APs are the DRAM tensors."""
    F32R = mybir.dt.float32r
    F32 = mybir.dt.float32
    AF = mybir.ActivationFunctionType

    with tc.tile_pool(name="persist", bufs=1) as pp:
        mask_sb = pp.tile([128, 256], F32, name="mask_sb")
        nc.sync.dma_start(mask_sb[:], mask)

        qkT = [pp.tile([128, _T], F32R, name=f"qkT{fg}") for fg in range(8)]
        v_pad = pp.tile([128, 8 * _VPH], F32R, name="v_pad")
        v_pad_h = v_pad[:].rearrange("p (h c) -> p h c", h=8)

        # ones columns of v_pad (col 64 of each 65-wide key-tile slot)
        v_pad_slots = v_pad[:].rearrange("p (h t c) -> p h t c", h=8, t=_NKT)
        for h in range(8):
            nc.vector.memset(v_pad_slots[:, h, :, 64].bitcast(F32), 1.0)

        # ---------------- Phase A: projections (T halved to fit SBUF) ------
        with tc.tile_pool(name="wpool", bufs=1) as wp, \
             tc.tile_pool(name="xtpool", bufs=10) as xp, \
             tc.tile_pool(name="papsum", bufs=4, space="PSUM") as pap:
            wqk_sb = [wp.tile([128, 1024], F32R, name=f"wqk{kc}") for kc in range(_NCT)]
            wv_sb = [wp.tile([128, _F], F32R, name=f"wv{kc}") for kc in range(_NCT)]
            for kc in range(_NCT):
                nc.sync.dma_start(wqk_sb[kc][:], wqk[128 * kc:128 * (kc + 1), :])
                nc.sync.dma_start(wv_sb[kc][:], wv[128 * kc:128 * (kc + 1), :])

            for half in range(2):
                xt_sb = []
                for kc in range(_NCT):
                    t = xp.tile([128, 1024], F32R, tag="xt", bufs=10, name=f"xt{half}_{kc}")
                    nc.sync.dma_start(
                        t[:], xt[128 * kc:128 * (kc + 1), 1024 * half:1024 * (half + 1)])
                    xt_sb.append(t)

                # qT / kT, feature-major: psum [128 feats, 512 T]
                for fg in range(8):
                    for chl in range(2):
                        ch = 2 * half + chl
                        ps = pap.tile([128, 512], F32, tag="pa", bufs=4, name=f"psqk{fg}_{ch}")
                        for kc in range(_NCT):
                            nc.tensor.matmul(
                                ps[:],
                                wqk_sb[kc][:, 128 * fg:128 * (fg + 1)],
                                xt_sb[kc][:, 512 * chl:512 * (chl + 1)],
                                start=(kc == 0), stop=(kc == _NCT - 1))
                        nc.vector.tensor_copy(qkT[fg][:, 512 * ch:512 * (ch + 1)], ps[:])

                # v natural: psum [128 T, 512 feats] -> strided into v_pad
                for ttl in range(8):
                    tt = 8 * half + ttl
                    ps = pap.tile([128, 512], F32, tag="pa", bufs=4, name=f"psv{tt}")
                    for kc in range(_NCT):
                        nc.tensor.matmul(
                            ps[:],
                            xt_sb[kc][:, 128 * ttl:128 * (ttl + 1)],
                            wv_sb[kc][:],
                            start=(kc == 0), stop=(kc == _NCT - 1))
                    nc.vector.tensor_copy(
                        v_pad_h[:, :, 65 * tt:65 * tt + 64],
                        ps[:].rearrange("p (h c) -> p h c", h=8))

        # ---------------- Phases B+C: attention + out_proj -----------------
        with tc.tile_pool(name="atpool", bufs=1) as ap_, \
             tc.tile_pool(name="wopool", bufs=1) as wop, \
             tc.tile_pool(name="ptpool", bufs=3) as ptp, \
             tc.tile_pool(name="bcpool", bufs=2) as bcp, \
             tc.tile_pool(name="ostpool", bufs=3) as osp, \
             tc.tile_pool(name="scpsum", bufs=2, space="PSUM") as scp, \
             tc.tile_pool(name="pvpsum", bufs=1, space="PSUM") as pvp, \
             tc.tile_pool(name="popsum", bufs=2, space="PSUM") as pop:

            attnT = [ap_.tile([128, _T], F32R, name=f"attnT{p}") for p in range(4)]
            wout_sb = [wop.tile([128, 1024], F32R, name=f"wout{f}") for f in range(4)]
            for f in range(4):
                nc.sync.dma_start(wout_sb[f][:], wout[128 * f:128 * (f + 1), :])

            for j in range(_NCH):
                for p in range(4):
                    pvA = pvp.tile([65, 512], F32, tag="pvA", bufs=1, name=f"pvA{j}_{p}")
                    pvB = pvp.tile([65, 512], F32, tag="pvB", bufs=1, name=f"pvB{j}_{p}")
                    nkt = 4 * (j + 1)

                    def col_lo_of(kt):
                        d = kt - 4 * j
                        if d < 0:
                            return 0
                        return 128 * d if d < 3 else 256

                    # stage 1 of the software pipeline: scores for one k-tile,
                    # both heads, into one 2-bank psum tile (A: cols 0:512,
                    # B: cols 512:1024).
                    def emit_scores(kt):
                        col_lo = col_lo_of(kt)
                        q_sl = slice(512 * j + col_lo, 512 * (j + 1))
                        k_sl = slice(128 * kt, 128 * (kt + 1))
                        ps2 = scp.tile([128, 1024], F32, tag="sc", bufs=2,
                                       name=f"ps2_{j}_{p}_{kt}")
                        nc.tensor.matmul(ps2[:, col_lo:512], qkT[4 + p][0:64, k_sl],
                                         qkT[p][0:64, q_sl], start=True, stop=True)
                        nc.tensor.matmul(ps2[:, 512 + col_lo:1024], qkT[4 + p][64:128, k_sl],
                                         qkT[p][64:128, q_sl], start=True, stop=True)
                        return ps2

                    # stage 2: one exp over both heads (2D strided AP), then
                    # boundary mask, then the two PV accumulation matmuls.
                    def emit_pv(kt, ps2):
                        col_lo = col_lo_of(kt)
                        d = kt - 4 * j
                        ncol = 512 - col_lo
                        pT2 = ptp.tile([128, 1024], F32R, tag="pT", bufs=3,
                                       name=f"pT2_{j}_{p}_{kt}")
                        src = ps2[:].rearrange("a (h c) -> a h c", h=2)[:, :, col_lo:]
                        dst = pT2[:].rearrange("a (h c) -> a h c", h=2)[:, :, col_lo:]
                        nc.scalar.activation(dst, src, AF.Exp, scale=0.125)
                        if d >= 0:
                            if d < 3:
                                m_sl = slice(128 * d, 128 * (d + 1))
                                msk = mask_sb[:, 128:256]
                                mw = 128
                            else:
                                m_sl = slice(256, 512)
                                msk = mask_sb[:, 0:256]
                                mw = 256
                            mdst = pT2[:].rearrange("a (h c) -> a h c", h=2)[:, :, m_sl]
                            nc.vector.tensor_mul(
                                mdst, mdst, msk[:, None, :].broadcast_to([128, 2, mw]))
                        c_sl = slice(col_lo, 512)
                        nc.tensor.matmul(pvA[0:65, c_sl],
                                         v_pad_h[:, 2 * p, 65 * kt:65 * kt + 65],
                                         pT2[:, col_lo:512],
                                         start=(kt == 0), stop=(kt == nkt - 1))
                        nc.tensor.matmul(pvB[0:65, c_sl],
                                         v_pad_h[:, 2 * p + 1, 65 * kt:65 * kt + 65],
                                         pT2[:, 512 + col_lo:1024],
                                         start=(kt == 0), stop=(kt == nkt - 1))

                    prev = emit_scores(0)
                    for kt in range(1, nkt):
                        cur = emit_scores(kt)
                        emit_pv(kt - 1, prev)
                        prev = cur
                    emit_pv(nkt - 1, prev)

                    # normalize: attnT rows = attn^T / denom
                    ch_sl = slice(512 * j, 512 * (j + 1))
                    recA = bcp.tile([1, 512], F32, tag="recA", bufs=2, name=f"recA{j}_{p}")
                    recB = bcp.tile([1, 512], F32, tag="recB", bufs=2, name=f"recB{j}_{p}")
                    nc.vector.reciprocal(recA[:], pvA[64:65, :])
                    nc.vector.reciprocal(recB[:], pvB[64:65, :])
                    bcA = bcp.tile([64, 512], F32, tag="bcA", bufs=2, name=f"bcA{j}_{p}")
                    bcB = bcp.tile([64, 512], F32, tag="bcB", bufs=2, name=f"bcB{j}_{p}")
                    nc.gpsimd.partition_broadcast(bcA[:], recA[:])
                    nc.gpsimd.partition_broadcast(bcB[:], recB[:])
                    nc.vector.tensor_mul(attnT[p][0:64, ch_sl], pvA[0:64, :], bcA[:])
                    nc.vector.tensor_mul(attnT[p][64:128, ch_sl], pvB[0:64, :], bcB[:])

                # out_proj for this chunk
                for mt in range(4):
                    t_sl = slice(512 * j + 128 * mt, 512 * j + 128 * (mt + 1))
                    ost = osp.tile([128, 1024], F32, tag="ost", bufs=3, name=f"ost{j}_{mt}")
                    for nh in range(2):
                        po = pop.tile([128, 512], F32, tag="po", bufs=1, name=f"po{j}_{mt}_{nh}")
                        for f in range(4):
                            nc.tensor.matmul(po[:],
                                             attnT[f][:, t_sl],
                                             wout_sb[f][:, 512 * nh:512 * (nh + 1)],
                                             start=(f == 0), stop=(f == 3))
                        nc.vector.tensor_copy(ost[:, 512 * nh:512 * (nh + 1)], po[:])
                    nc.sync.dma_start(out[t_sl, :], ost[:])


def build_program(repeat=1):
    import concourse.bass as bass  # noqa: F401
    import concourse.mybir as mybir
    import concourse.tile as tile
    from concourse import bacc

    F32R = mybir.dt.float32r
    F32 = mybir.dt.float32

    nc = bacc.Bacc("TRN2", target_bir_lowering=False, debug=False,
                   num_devices=N_CORES)
    xt = nc.dram_tensor("xt", [_C, _T], F32R, kind="ExternalInput").ap()
    wqk = nc.dram_tensor("wqk", [_C, 2 * _F], F32R, kind="ExternalInput").ap()
    wv = nc.dram_tensor("wv", [_C, _F], F32R, kind="ExternalInput").ap()
    wout = nc.dram_tensor("wout", [_F, _C], F32R, kind="ExternalInput").ap()
    mask = nc.dram_tensor("mask", [128, 256], F32, kind="ExternalInput").ap()
    out = nc.dram_tensor("out", [_T, _C], F32, kind="ExternalOutput").ap()

    with tile.TileContext(nc) as tc:
        if repeat == 1:
            _emit_body(nc, tc, mybir, xt, wqk, wv, wout, mask, out)
        else:
            with tc.For_i(0, repeat, 1):
                _emit_body(nc, tc, mybir, xt, wqk, wv, wout, mask, out)
    nc.compile()
    return nc


def make_in_maps(x, w_qkv, w_out):
    """Host-side sharding: per-core input dicts."""
    x = np.asarray(x, dtype=np.float32)
    w_qkv = np.asarray(w_qkv, dtype=np.float32)
    w_out = np.asarray(w_out, dtype=np.float32)

    mask_np = np.zeros((128, 256), np.float32)
    mask_np[:, 128:] = (np.arange(128)[:, None] <= np.arange(128)[None, :])

    xts = [np.ascontiguousarray(x[b].T) for b in range(_B)]
    in_maps = []
    for c in range(N_CORES):
        b, g = divmod(c, 2)
        qcols = w_qkv[:, 512 * g:512 * (g + 1)]
        kcols = w_qkv[:, _C + 512 * g:_C + 512 * (g + 1)]
        vcols = w_qkv[:, 2 * _C + 512 * g:2 * _C + 512 * (g + 1)]
        in_maps.append({
            "xt": xts[b],
            "wqk": np.ascontiguousarray(np.concatenate([qcols, kcols], axis=1)),
            "wv": np.ascontiguousarray(vcols),
            "wout": np.ascontiguousarray(w_out[512 * g:512 * (g + 1), :]),
            "mask": mask_np,
        })
    return in_maps


_PROGRAM_CACHE = {}


def _get_program(repeat=1):
    if repeat not in _PROGRAM_CACHE:
        _PROGRAM_CACHE[repeat] = build_program(repeat)
    return _PROGRAM_CACHE[repeat]


def kernel(x, w_qkv, w_out):
    from concourse.bass_utils import run_bass_kernel_spmd

    nc = _get_program(1)
    in_maps = make_in_maps(x, w_qkv, w_out)
    res = run_bass_kernel_spmd(nc, in_maps, core_ids=list(range(N_CORES)))
    out = np.empty((_B, _T, _C), np.float32)
    for b in range(_B):
        np.add(res.results[2 * b]["out"], res.results[2 * b + 1]["out"], out=out[b])
    return out


# revision 9
# speedup vs baseline: 1.4161x; 1.1917x over previous
# Multi-head causal self-attention (B=4, T=2048, C=1024, H=16) on 8 NeuronCores.
#
# Sharding: core c handles batch b = c//2 and head-group g = c%2 (8 heads each).
# Each core computes q/k/v projections for its 8 heads, causal flash-style
# attention, and a partial out-projection (its 512 rows of w_out). The host
# sums the two partial outputs per batch (the 2-way tensor-parallel
# all-reduce) and concatenates batches.
#
# On-device layout choices (all matmuls run as fp32r = fp22-truncated fp32,
# full PE rate at moving dim >= 256):
#   - x is pre-transposed on host: xt [C, T] so the C-contraction of the
#     projections has C on SBUF partitions.
#   - q,k are produced feature-major (qT/kT [64*heads, T]) so scores can be
#     computed transposed: S^T [T_k, T_q] = kT.T @ qT, K=64, two heads packed
#     into the PE array via row tile_position (0/64).
#   - softmax runs without max-subtraction (scores are O(5), exp is safe in
#     fp32): P^T = exp(S^T/8) via ScalarE directly PSUM->SBUF.
#   - v is produced natural ([T, feats]) into v_pad with a ones-column per
#     128-key tile, so the PV matmul (V|1).T @ P^T accumulates both attn^T and
#     the softmax denominators in one pass: psum [65, T_q].
#   - causal masking: per diagonal 128-key tile only cols >= 128*d are
#     computed (restricted matmul output range); the remaining 128/256-wide
#     boundary block is masked by one DVE multiply with a host-built
#     triangular mask.
#   - normalization is fused into the PSUM->SBUF copy of attn^T (multiply by
#     the partition-broadcast reciprocal of the denominator row).
#   - out_proj: out[T,1024] partial = attnT.T @ w_out[512 rows], accumulated
#     over 4 feature tiles.

import numpy as np

_B, _T, _C = 4, 2048, 1024
_H, _DK = 16, 64
N_CORES = 8
_F = 512            # local features per core (8 heads x 64)
_NCT = _C // 128    # 8 C tiles
_NKT = _T // 128    # 16 key tiles
_NCH = _T // 512    # 4 query chunks of 512
_VPH = _NKT * 65    # 1040 v_pad cols per head


def _emit_body(nc, tc, mybir, xt, wqk, wv, wout, mask, out, phases="ABC"):
    """Emit one full forward pass. APs are the DRAM tensors."""
    F32R = mybir.dt.float32r
    F32 = mybir.dt.float32
    AF = mybir.ActivationFunctionType

    with tc.tile_pool(name="persist", bufs=1) as pp:
        mask_sb = pp.tile([128, 256], F32, name="mask_sb")
        nc.sync.dma_start(mask_sb[:], mask)

        qkT = [pp.tile([128, _T], F32R, name=f"qkT{fg}") for fg in range(8)]
        v_pad = pp.tile([128, 8 * _VPH], F32R, name="v_pad")
        v_pad_h = v_pad[:].rearrange("p (h c) -> p h c", h=8)

        # ones columns of v_pad (col 64 of each 65-wide key-tile slot)
        v_pad_slots = v_pad[:].rearrange("p (h t c) -> p h t c", h=8, t=_NKT)
        for h in range(8):
            nc.vector.memset(v_pad_slots[:, h, :, 64].bitcast(F32), 1.0)

        # ---------------- Phase A: projections (T halved to fit SBUF) ------
        with tc.tile_pool(name="wpool", bufs=1) as wp, \
             tc.tile_pool(name="xtpool", bufs=10) as xp, \
             tc.tile_pool(name="papsum", bufs=4, space="PSUM") as pap:
            wqk_sb = [wp.tile([128, 1024], F32R, name=f"wqk{kc}") for kc in range(_NCT)]
            wv_sb = [wp.tile([128, _F], F32R, name=f"wv{kc}") for kc in range(_NCT)]
            for kc in range(_NCT):
                nc.sync.dma_start(wqk_sb[kc][:], wqk[128 * kc:128 * (kc + 1), :])
                nc.sync.dma_start(wv_sb[kc][:], wv[128 * kc:128 * (kc + 1), :])

            for half in range(2):
                xt_sb = []
                for kc in range(_NCT):
                    t = xp.tile([128, 1024], F32R, tag="xt", bufs=10, name=f"xt{half}_{kc}")
                    nc.sync.dma_start(
                        t[:], xt[128 * kc:128 * (kc + 1), 1024 * half:1024 * (half + 1)])
                    xt_sb.append(t)

                # qT / kT, feature-major: psum [128 feats, 512 T]
                for fg in range(8):
                    for chl in range(2):
                        ch = 2 * half + chl
                        ps = pap.tile([128, 512], F32, tag="pa", bufs=4, name=f"psqk{fg}_{ch}")
                        for kc in range(_NCT):
                            nc.tensor.matmul(
                                ps[:],
                                wqk_sb[kc][:, 128 * fg:128 * (fg + 1)],
                                xt_sb[kc][:, 512 * chl:512 * (chl + 1)],
                                start=(kc == 0), stop=(kc == _NCT - 1))
                        nc.vector.tensor_copy(qkT[fg][:, 512 * ch:512 * (ch + 1)], ps[:])

                # v natural: psum [128 T, 512 feats] -> strided into v_pad
                for ttl in range(8):
                    tt = 8 * half + ttl
                    ps = pap.tile([128, 512], F32, tag="pa", bufs=4, name=f"psv{tt}")
                    for kc in range(_NCT):
                        nc.tensor.matmul(
                            ps[:],
                            xt_sb[kc][:, 128 * ttl:128 * (ttl + 1)],
                            wv_sb[kc][:],
                            start=(kc == 0), stop=(kc == _NCT - 1))
                    nc.vector.tensor_copy(
                        v_pad_h[:, :, 65 * tt:65 * tt + 64],
                        ps[:].rearrange("p (h c) -> p h c", h=8))

        # ---------------- Phases B+C: attention + out_proj -----------------
        if "B" not in phases:
            return
        with tc.tile_pool(name="atpool", bufs=1) as ap_, \
             tc.tile_pool(name="wopool", bufs=1) as wop, \
             tc.tile_pool(name="ptpool", bufs=3) as ptp, \
             tc.tile_pool(name="bcpool", bufs=2) as bcp, \
             tc.tile_pool(name="ostpool", bufs=3) as osp, \
             tc.tile_pool(name="scpsum", bufs=2, space="PSUM") as scp, \
             tc.tile_pool(name="pvpsum", bufs=1, space="PSUM") as pvp, \
             tc.tile_pool(name="popsum", bufs=2, space="PSUM") as pop:

            attnT = [ap_.tile([128, _T], F32R, name=f"attnT{p}") for p in range(4)]
            wout_sb = [wop.tile([128, 1024], F32R, name=f"wout{f}") for f in range(4)]
            for f in range(4):
                nc.sync.dma_start(wout_sb[f][:], wout[128 * f:128 * (f + 1), :])

            for j in range(_NCH):
                for p in range(4):
                    pvA = pvp.tile([65, 512], F32, tag="pvA", bufs=1, name=f"pvA{j}_{p}")
                    pvB = pvp.tile([65, 512], F32, tag="pvB", bufs=1, name=f"pvB{j}_{p}")
                    nkt = 4 * (j + 1)

                    def col_lo_of(kt):
                        d = kt - 4 * j
                        if d < 0:
                            return 0
                        return 128 * d if d < 3 else 256

                    # stage 1 of the software pipeline: scores for one k-tile,
                    # both heads, into one 2-bank psum tile (A: cols 0:512,
                    # B: cols 512:1024).
                    def emit_scores(kt):
                        col_lo = col_lo_of(kt)
                        q_sl = slice(512 * j + col_lo, 512 * (j + 1))
                        k_sl = slice(128 * kt, 128 * (kt + 1))
                        ps2 = scp.tile([128, 1024], F32, tag="sc", bufs=2,
                                       name=f"ps2_{j}_{p}_{kt}")
                        nc.tensor.matmul(ps2[:, col_lo:512], qkT[4 + p][0:64, k_sl],
                                         qkT[p][0:64, q_sl], start=True, stop=True)
                        nc.tensor.matmul(ps2[:, 512 + col_lo:1024], qkT[4 + p][64:128, k_sl],
                                         qkT[p][64:128, q_sl], start=True, stop=True)
                        return ps2

                    # stage 2: one exp over both heads (2D strided AP), then
                    # boundary mask, then the two PV accumulation matmuls.
                    def emit_pv(kt, ps2):
                        col_lo = col_lo_of(kt)
                        d = kt - 4 * j
                        ncol = 512 - col_lo
                        pT2 = ptp.tile([128, 1024], F32R, tag="pT", bufs=3,
                                       name=f"pT2_{j}_{p}_{kt}")
                        src = ps2[:].rearrange("a (h c) -> a h c", h=2)[:, :, col_lo:]
                        dst = pT2[:].rearrange("a (h c) -> a h c", h=2)[:, :, col_lo:]
                        nc.scalar.activation(dst, src, AF.Exp, scale=0.125)
                        if d >= 0:
                            if d < 3:
                                m_sl = slice(128 * d, 128 * (d + 1))
                                msk = mask_sb[:, 128:256]
                                mw = 128
                            else:
                                m_sl = slice(256, 512)
                                msk = mask_sb[:, 0:256]
                                mw = 256
                            mdst = pT2[:].rearrange("a (h c) -> a h c", h=2)[:, :, m_sl]
                            nc.vector.tensor_mul(
                                mdst, mdst, msk[:, None, :].broadcast_to([128, 2, mw]))
                        c_sl = slice(col_lo, 512)
                        nc.tensor.matmul(pvA[0:65, c_sl],
                                         v_pad_h[:, 2 * p, 65 * kt:65 * kt + 65],
                                         pT2[:, col_lo:512],
                                         start=(kt == 0), stop=(kt == nkt - 1))
                        nc.tensor.matmul(pvB[0:65, c_sl],
                                         v_pad_h[:, 2 * p + 1, 65 * kt:65 * kt + 65],
                                         pT2[:, 512 + col_lo:1024],
                                         start=(kt == 0), stop=(kt == nkt - 1))

                    prev = emit_scores(0)
                    for kt in range(1, nkt):
                        cur = emit_scores(kt)
                        emit_pv(kt - 1, prev)
                        prev = cur
                    emit_pv(nkt - 1, prev)

                    # normalize: attnT rows = attn^T / denom
                    ch_sl = slice(512 * j, 512 * (j + 1))
                    recA = bcp.tile([1, 512], F32, tag="recA", bufs=2, name=f"recA{j}_{p}")
                    recB = bcp.tile([1, 512], F32, tag="recB", bufs=2, name=f"recB{j}_{p}")
                    nc.vector.reciprocal(recA[:], pvA[64:65, :])
                    nc.vector.reciprocal(recB[:], pvB[64:65, :])
                    bcA = bcp.tile([64, 512], F32, tag="bcA", bufs=2, name=f"bcA{j}_{p}")
                    bcB = bcp.tile([64, 512], F32, tag="bcB", bufs=2, name=f"bcB{j}_{p}")
                    nc.gpsimd.partition_broadcast(bcA[:], recA[:])
                    nc.gpsimd.partition_broadcast(bcB[:], recB[:])
                    nc.vector.tensor_mul(attnT[p][0:64, ch_sl], pvA[0:64, :], bcA[:])
                    nc.vector.tensor_mul(attnT[p][64:128, ch_sl], pvB[0:64, :], bcB[:])

                # out_proj for this chunk
                if "C" not in phases:
                    continue
                for mt in range(4):
                    t_sl = slice(512 * j + 128 * mt, 512 * j + 128 * (mt + 1))
                    ost = osp.tile([128, 1024], F32, tag="ost", bufs=3, name=f"ost{j}_{mt}")
                    for nh in range(2):
                        po = pop.tile([128, 512], F32, tag="po", bufs=1, name=f"po{j}_{mt}_{nh}")
                        for f in range(4):
                            nc.tensor.matmul(po[:],
                                             attnT[f][:, t_sl],
                                             wout_sb[f][:, 512 * nh:512 * (nh + 1)],
                                             start=(f == 0), stop=(f == 3))
                        nc.vector.tensor_copy(ost[:, 512 * nh:512 * (nh + 1)], po[:])
                    nc.sync.dma_start(out[t_sl, :], ost[:])


def build_program(repeat=1, phases="ABC"):
    import concourse.bass as bass  # noqa: F401
    import concourse.mybir as mybir
    import concourse.tile as tile
    from concourse import bacc

    F32R = mybir.dt.float32r
    F32 = mybir.dt.float32

    nc = bacc.Bacc("TRN2", target_bir_lowering=False, debug=False,
                   num_devices=N_CORES)
    xt = nc.dram_tensor("xt", [_C, _T], F32R, kind="ExternalInput").ap()
    wqk = nc.dram_tensor("wqk", [_C, 2 * _F], F32R, kind="ExternalInput").ap()
    wv = nc.dram_tensor("wv", [_C, _F], F32R, kind="ExternalInput").ap()
    wout = nc.dram_tensor("wout", [_F, _C], F32R, kind="ExternalInput").ap()
    mask = nc.dram_tensor("mask", [128, 256], F32, kind="ExternalInput").ap()
    out = nc.dram_tensor("out", [_T, _C], F32, kind="ExternalOutput").ap()

    with tile.TileContext(nc) as tc:
        if repeat == 1:
            _emit_body(nc, tc, mybir, xt, wqk, wv, wout, mask, out, phases)
        else:
            with tc.For_i(0, repeat, 1):
                _emit_body(nc, tc, mybir, xt, wqk, wv, wout, mask, out, phases)
    nc.compile()
    return nc


def make_in_maps(x, w_qkv, w_out):
    """Host-side sharding: per-core input dicts."""
    x = np.asarray(x, dtype=np.float32)
    w_qkv = np.asarray(w_qkv, dtype=np.float32)
    w_out = np.asarray(w_out, dtype=np.float32)

    mask_np = np.zeros((128, 256), np.float32)
    mask_np[:, 128:] = (np.arange(128)[:, None] <= np.arange(128)[None, :])

    xts = [np.ascontiguousarray(x[b].T) for b in range(_B)]
    in_maps = []
    for c in range(N_CORES):
        b, g = divmod(c, 2)
        qcols = w_qkv[:, 512 * g:512 * (g + 1)]
        kcols = w_qkv[:, _C + 512 * g:_C + 512 * (g + 1)]
        vcols = w_qkv[:, 2 * _C + 512 * g:2 * _C + 512 * (g + 1)]
        in_maps.append({
            "xt": xts[b],
            "wqk": np.ascontiguousarray(np.concatenate([qcols, kcols], axis=1)),
            "wv": np.ascontiguousarray(vcols),
            "wout": np.ascontiguousarray(w_out[512 * g:512 * (g + 1), :]),
            "mask": mask_np,
        })
    return in_maps


_PROGRAM_CACHE = {}


def _get_program(repeat=1):
    if repeat not in _PROGRAM_CACHE:
        _PROGRAM_CACHE[repeat] = build_program(repeat)
    return _PROGRAM_CACHE[repeat]


def kernel(x, w_qkv, w_out):
    from concourse.bass_utils import run_bass_kernel_spmd

    nc = _get_program(1)
    in_maps = make_in_maps(x, w_qkv, w_out)
    res = run_bass_kernel_spmd(nc, in_maps, core_ids=list(range(N_CORES)))
    out = np.empty((_B, _T, _C), np.float32)
    for b in range(_B):
        np.add(res.results[2 * b]["out"], res.results[2 * b + 1]["out"], out=out[b])
    return out


# revision 14
# speedup vs baseline: 11.2863x; 7.9698x over previous
# Multi-head causal self-attention (B=4, T=2048, C=1024, H=16) on 8 NeuronCores.
#
# Sharding: core c handles batch b = c//2 and head-group g = c%2 (8 heads each).
# Each core computes q/k/v projections for its 8 heads, causal flash-style
# attention, and a partial out-projection (its 512 rows of w_out). The host
# sums the two partial outputs per batch (the 2-way tensor-parallel
# all-reduce) and concatenates batches.
#
# On-device layout choices (all matmuls run as fp32r = fp22-truncated fp32,
# full PE rate at moving dim >= 256):
#   - x is pre-transposed on host: xt [C, T] so the C-contraction of the
#     projections has C on SBUF partitions.
#   - q,k are produced feature-major (qT/kT [64*heads, T]) so scores can be
#     computed transposed: S^T [T_k, T_q] = kT.T @ qT, K=64, two heads packed
#     into the PE array via row tile_position (0/64).
#   - softmax runs without max-subtraction (scores are O(5), exp is safe in
#     fp32): P^T = exp(S^T/8) via ScalarE directly PSUM->SBUF.
#   - v is produced natural ([T, feats]) into v_pad with a ones-column per
#     128-key tile, so the PV matmul (V|1).T @ P^T accumulates both attn^T and
#     the softmax denominators in one pass: psum [65, T_q].
#   - causal masking: per diagonal 128-key tile only cols >= 128*d are
#     computed (restricted matmul output range); the remaining 128/256-wide
#     boundary block is masked by one DVE multiply with a host-built
#     triangular mask.
#   - normalization is fused into the PSUM->SBUF copy of attn^T (multiply by
#     the partition-broadcast reciprocal of the denominator row).
#   - out_proj: out[T,1024] partial = attnT.T @ w_out[512 rows], accumulated
#     over 4 feature tiles.

import numpy as np

_B, _T, _C = 4, 2048, 1024
_H, _DK = 16, 64
N_CORES = 8
_F = 512            # local features per core (8 heads x 64)
_NCT = _C // 128    # 8 C tiles
_NKT = _T // 128    # 16 key tiles
_NCH = _T // 512    # 4 query chunks of 512
_VPH = _NKT * 65    # 1040 v_pad cols per head


def _emit_body(nc, tc, mybir, xt, wqk, wv, wout, mask, out, phases="ABC",
               a_dtype=None):
    """Emit one full forward pass. APs are the DRAM tensors."""
    F32R = a_dtype or mybir.dt.float32r
    F32 = mybir.dt.float32
    AF = mybir.ActivationFunctionType

    with tc.tile_pool(name="persist", bufs=1) as pp:
        mask_sb = pp.tile([128, 256], F32, name="mask_sb")
        nc.sync.dma_start(mask_sb[:], mask)

        qkT = [pp.tile([128, _T], F32R, name=f"qkT{fg}") for fg in range(8)]
        v_pad = pp.tile([128, 8 * _VPH], F32R, name="v_pad")
        v_pad_h = v_pad[:].rearrange("p (h c) -> p h c", h=8)

        # ones columns of v_pad (col 64 of each 65-wide key-tile slot)
        v_pad_slots = v_pad[:].rearrange("p (h t c) -> p h t c", h=8, t=_NKT)
        for h in range(8):
            ones_ap = v_pad_slots[:, h, :, 64]
            if F32R == mybir.dt.float32r:
                ones_ap = ones_ap.bitcast(F32)
            nc.vector.memset(ones_ap, 1.0)

        # ---------------- Phase A: projections (T halved to fit SBUF) ------
        with tc.tile_pool(name="wpool", bufs=1) as wp, \
             tc.tile_pool(name="xtpool", bufs=10) as xp, \
             tc.tile_pool(name="papsum", bufs=4, space="PSUM") as pap:
            wqk_sb = [wp.tile([128, 1024], F32R, name=f"wqk{kc}") for kc in range(_NCT)]
            wv_sb = [wp.tile([128, _F], F32R, name=f"wv{kc}") for kc in range(_NCT)]
            for kc in range(_NCT):
                nc.sync.dma_start(wqk_sb[kc][:], wqk[128 * kc:128 * (kc + 1), :])
                nc.sync.dma_start(wv_sb[kc][:], wv[128 * kc:128 * (kc + 1), :])

            for half in range(2):
                xt_sb = []
                for kc in range(_NCT):
                    t = xp.tile([128, 1024], F32R, tag="xt", bufs=10, name=f"xt{half}_{kc}")
                    nc.sync.dma_start(
                        t[:], xt[128 * kc:128 * (kc + 1), 1024 * half:1024 * (half + 1)])
                    xt_sb.append(t)

                # qT / kT, feature-major: psum [128 feats, 512 T]
                for fg in range(8):
                    for chl in range(2):
                        ch = 2 * half + chl
                        ps = pap.tile([128, 512], F32, tag="pa", bufs=4, name=f"psqk{fg}_{ch}")
                        for kc in range(_NCT):
                            nc.tensor.matmul(
                                ps[:],
                                wqk_sb[kc][:, 128 * fg:128 * (fg + 1)],
                                xt_sb[kc][:, 512 * chl:512 * (chl + 1)],
                                start=(kc == 0), stop=(kc == _NCT - 1))
                        nc.vector.tensor_copy(qkT[fg][:, 512 * ch:512 * (ch + 1)], ps[:])

                # v natural: psum [128 T, 512 feats] -> strided into v_pad
                for ttl in range(8):
                    tt = 8 * half + ttl
                    ps = pap.tile([128, 512], F32, tag="pa", bufs=4, name=f"psv{tt}")
                    for kc in range(_NCT):
                        nc.tensor.matmul(
                            ps[:],
                            xt_sb[kc][:, 128 * ttl:128 * (ttl + 1)],
                            wv_sb[kc][:],
                            start=(kc == 0), stop=(kc == _NCT - 1))
                    nc.vector.tensor_copy(
                        v_pad_h[:, :, 65 * tt:65 * tt + 64],
                        ps[:].rearrange("p (h c) -> p h c", h=8))

        # ---------------- Phases B+C: attention + out_proj -----------------
        if "B" not in phases:
            return
        with tc.tile_pool(name="atpool", bufs=1) as ap_, \
             tc.tile_pool(name="wopool", bufs=1) as wop, \
             tc.tile_pool(name="ptpool", bufs=3) as ptp, \
             tc.tile_pool(name="bcpool", bufs=2) as bcp, \
             tc.tile_pool(name="ostpool", bufs=3) as osp, \
             tc.tile_pool(name="scpsum", bufs=2, space="PSUM") as scp, \
             tc.tile_pool(name="pvpsum", bufs=1, space="PSUM") as pvp, \
             tc.tile_pool(name="popsum", bufs=2, space="PSUM") as pop:

            attnT = [ap_.tile([128, _T], F32R, name=f"attnT{p}") for p in range(4)]
            wout_sb = [wop.tile([128, 1024], F32R, name=f"wout{f}") for f in range(4)]
            for f in range(4):
                nc.sync.dma_start(wout_sb[f][:], wout[128 * f:128 * (f + 1), :])

            for j in range(_NCH):
                for p in range(4):
                    pvA = pvp.tile([65, 512], F32, tag="pvA", bufs=1, name=f"pvA{j}_{p}")
                    pvB = pvp.tile([65, 512], F32, tag="pvB", bufs=1, name=f"pvB{j}_{p}")
                    nkt = 4 * (j + 1)

                    def col_lo_of(kt):
                        d = kt - 4 * j
                        if d < 0:
                            return 0
                        return 128 * d if d < 3 else 256

                    # stage 1 of the software pipeline: scores for one k-tile,
                    # both heads, into one 2-bank psum tile (A: cols 0:512,
                    # B: cols 512:1024).
                    def emit_scores(kt):
                        col_lo = col_lo_of(kt)
                        q_sl = slice(512 * j + col_lo, 512 * (j + 1))
                        k_sl = slice(128 * kt, 128 * (kt + 1))
                        ps2 = scp.tile([128, 1024], F32, tag="sc", bufs=2,
                                       name=f"ps2_{j}_{p}_{kt}")
                        nc.tensor.matmul(ps2[:, col_lo:512], qkT[4 + p][0:64, k_sl],
                                         qkT[p][0:64, q_sl], start=True, stop=True)
                        nc.tensor.matmul(ps2[:, 512 + col_lo:1024], qkT[4 + p][64:128, k_sl],
                                         qkT[p][64:128, q_sl], start=True, stop=True)
                        return ps2

                    # stage 2: one exp over both heads (2D strided AP), then
                    # boundary mask, then the two PV accumulation matmuls.
                    def emit_pv(kt, ps2):
                        col_lo = col_lo_of(kt)
                        d = kt - 4 * j
                        ncol = 512 - col_lo
                        pT2 = ptp.tile([128, 1024], F32R, tag="pT", bufs=3,
                                       name=f"pT2_{j}_{p}_{kt}")
                        src = ps2[:].rearrange("a (h c) -> a h c", h=2)[:, :, col_lo:]
                        dst = pT2[:].rearrange("a (h c) -> a h c", h=2)[:, :, col_lo:]
                        nc.scalar.activation(dst, src, AF.Exp, scale=0.125)
                        if d >= 0:
                            if d < 3:
                                m_sl = slice(128 * d, 128 * (d + 1))
                                msk = mask_sb[:, 128:256]
                                mw = 128
                            else:
                                m_sl = slice(256, 512)
                                msk = mask_sb[:, 0:256]
                                mw = 256
                            mdst = pT2[:].rearrange("a (h c) -> a h c", h=2)[:, :, m_sl]
                            nc.vector.tensor_mul(
                                mdst, mdst, msk[:, None, :].broadcast_to([128, 2, mw]))
                        c_sl = slice(col_lo, 512)
                        nc.tensor.matmul(pvA[0:65, c_sl],
                                         v_pad_h[:, 2 * p, 65 * kt:65 * kt + 65],
                                         pT2[:, col_lo:512],
                                         start=(kt == 0), stop=(kt == nkt - 1))
                        nc.tensor.matmul(pvB[0:65, c_sl],
                                         v_pad_h[:, 2 * p + 1, 65 * kt:65 * kt + 65],
                                         pT2[:, 512 + col_lo:1024],
                                         start=(kt == 0), stop=(kt == nkt - 1))

                    prev = emit_scores(0)
                    for kt in range(1, nkt):
                        cur = emit_scores(kt)
                        emit_pv(kt - 1, prev)
                        prev = cur
                    emit_pv(nkt - 1, prev)

                    # normalize: attnT rows = attn^T / denom
                    ch_sl = slice(512 * j, 512 * (j + 1))
                    recA = bcp.tile([1, 512], F32, tag="recA", bufs=2, name=f"recA{j}_{p}")
                    recB = bcp.tile([1, 512], F32, tag="recB", bufs=2, name=f"recB{j}_{p}")
                    nc.vector.reciprocal(recA[:], pvA[64:65, :])
                    nc.vector.reciprocal(recB[:], pvB[64:65, :])
                    bcA = bcp.tile([64, 512], F32, tag="bcA", bufs=2, name=f"bcA{j}_{p}")
                    bcB = bcp.tile([64, 512], F32, tag="bcB", bufs=2, name=f"bcB{j}_{p}")
                    nc.gpsimd.partition_broadcast(bcA[:], recA[:])
                    nc.gpsimd.partition_broadcast(bcB[:], recB[:])
                    nc.vector.tensor_mul(attnT[p][0:64, ch_sl], pvA[0:64, :], bcA[:])
                    nc.vector.tensor_mul(attnT[p][64:128, ch_sl], pvB[0:64, :], bcB[:])

                # out_proj for this chunk
                if "C" not in phases:
                    continue
                for mt in range(4):
                    t_sl = slice(512 * j + 128 * mt, 512 * j + 128 * (mt + 1))
                    ost = osp.tile([128, 1024], F32, tag="ost", bufs=3, name=f"ost{j}_{mt}")
                    for nh in range(2):
                        po = pop.tile([128, 512], F32, tag="po", bufs=1, name=f"po{j}_{mt}_{nh}")
                        for f in range(4):
                            nc.tensor.matmul(po[:],
                                             attnT[f][:, t_sl],
                                             wout_sb[f][:, 512 * nh:512 * (nh + 1)],
                                             start=(f == 0), stop=(f == 3))
                        nc.vector.tensor_copy(ost[:, 512 * nh:512 * (nh + 1)], po[:])
                    nc.sync.dma_start(out[t_sl, :], ost[:])


def build_program(repeat=1, phases="ABC", a_dtype_name=None):
    import concourse.bass as bass  # noqa: F401
    import concourse.mybir as mybir
    import concourse.tile as tile
    from concourse import bacc

    a_dtype = getattr(mybir.dt, a_dtype_name) if a_dtype_name else None
    F32R = a_dtype or mybir.dt.float32r
    F32 = mybir.dt.float32

    nc = bacc.Bacc("TRN2", target_bir_lowering=False, debug=False,
                   num_devices=N_CORES)
    xt = nc.dram_tensor("xt", [_C, _T], F32R, kind="ExternalInput").ap()
    wqk = nc.dram_tensor("wqk", [_C, 2 * _F], F32R, kind="ExternalInput").ap()
    wv = nc.dram_tensor("wv", [_C, _F], F32R, kind="ExternalInput").ap()
    wout = nc.dram_tensor("wout", [_F, _C], F32R, kind="ExternalInput").ap()
    mask = nc.dram_tensor("mask", [128, 256], F32, kind="ExternalInput").ap()
    out = nc.dram_tensor("out", [_T, _C], F32, kind="ExternalOutput").ap()

    with tile.TileContext(nc) as tc:
        if repeat == 1:
            _emit_body(nc, tc, mybir, xt, wqk, wv, wout, mask, out, phases, a_dtype)
        else:
            with tc.For_i(0, repeat, 1):
                _emit_body(nc, tc, mybir, xt, wqk, wv, wout, mask, out, phases, a_dtype)
    nc.compile()
    return nc


def make_in_maps(x, w_qkv, w_out, a_dtype_name=None):
    """Host-side sharding: per-core input dicts."""
    if a_dtype_name == "bfloat16":
        import ml_dtypes
        adt = ml_dtypes.bfloat16
    else:
        adt = np.float32
    x = np.asarray(x, dtype=adt)
    w_qkv = np.asarray(w_qkv, dtype=adt)
    w_out = np.asarray(w_out, dtype=adt)

    mask_np = np.zeros((128, 256), np.float32)
    mask_np[:, 128:] = (np.arange(128)[:, None] <= np.arange(128)[None, :])

    xts = [np.ascontiguousarray(x[b].T) for b in range(_B)]
    in_maps = []
    for c in range(N_CORES):
        b, g = divmod(c, 2)
        qcols = w_qkv[:, 512 * g:512 * (g + 1)]
        kcols = w_qkv[:, _C + 512 * g:_C + 512 * (g + 1)]
        vcols = w_qkv[:, 2 * _C + 512 * g:2 * _C + 512 * (g + 1)]
        in_maps.append({
            "xt": xts[b],
            "wqk": np.ascontiguousarray(np.concatenate([qcols, kcols], axis=1)),
            "wv": np.ascontiguousarray(vcols),
            "wout": np.ascontiguousarray(w_out[512 * g:512 * (g + 1), :]),
            "mask": mask_np,
        })
    return in_maps


_PROGRAM_CACHE = {}


def _get_program(repeat=1):
    if repeat not in _PROGRAM_CACHE:
        _PROGRAM_CACHE[repeat] = build_program(repeat)
    return _PROGRAM_CACHE[repeat]


def kernel(x, w_qkv, w_out):
    from concourse.bass_utils import run_bass_kernel_spmd

    nc = _get_program(1)
    in_maps = make_in_maps(x, w_qkv, w_out)
    res = run_bass_kernel_spmd(nc, in_maps, core_ids=list(range(N_CORES)))
    out = np.empty((_B, _T, _C), np.float32)
    for b in range(_B):
        np.add(res.results[2 * b]["out"], res.results[2 * b + 1]["out"], out=out[b])
    return out
